# revision 6
# baseline (speedup 1.0000x reference)
"""DLRM-top kernel for 8 TRN2 NeuronCores (data-parallel over batch).

Pipeline per core (4096 samples, tiles of NB):
  1. gpsimd cast-DMA loads x tile f32->fp16 into stage [112, G*128]
     (partition = 32*j + n for 4 samples j per group, pitch-32 junk rows).
  2. xbar DMA-transpose per group: stage[:, g*128:+128] -> XT[:, g*128:+128]
     giving d-major layout XT[d, g*128 + 32j + n] = x[4g+j, n, d].
  3. Per-sample gram matmuls (fp16): psum[0:27, sl*27:+27] = Xs @ Xs.T.
  4. ACT copies psum -> S_part[n, s*27 + m] (fp16).
  5. DVE copies build FI k-tiles [128, NB]: pair (m, n>m) at partition
     (m%4)*32 + n of k-tile m//4 (garbage rows masked by zero weights).
  6. Feature-major MLP, batch on free dim (N=NB), fp16 weights with FWL.
"""

import sys

sys.path.insert(0, "/opt/trn_rl_repo")

import numpy as np

_B = 32768
_NCORES = 8
_BC = _B // _NCORES  # 4096
_NB = 256  # samples per tile
_G = _NB // 4  # transpose groups per tile
_N = 27
_D = 128
_NPAIR = _N * (_N - 1) // 2  # 351
_NKT = 7  # FI k-tiles (27 m-blocks in 32-row slots, 4 per tile)
_SPB = 18  # gram samples per psum bank (18*27=486 <= 512)

_compiled = {}


def _build(nb, nt):
    import concourse.bass as bass
    import concourse.mybir as mybir
    import concourse.tile as tile
    from concourse import bacc

    f16 = mybir.dt.float16
    f32 = mybir.dt.float32
    Relu = mybir.ActivationFunctionType.Relu

    g = nb // 4
    nbanks = (nb + _SPB - 1) // _SPB  # gram psum banks per tile

    nc = bacc.Bacc("TRN2", target_bir_lowering=False, debug=False,
                   num_devices=_NCORES)

    x = nc.dram_tensor("x", [nb * nt * _N * _D + 8 * _D], f32, kind="ExternalInput")
    y = nc.dram_tensor("y", [nb * nt, 1], f32, kind="ExternalOutput")
    w0bT = nc.dram_tensor("w0bT", [128, 1024], f16, kind="ExternalInput")
    w0p = nc.dram_tensor("w0p", [128, _NKT * 1024], f16, kind="ExternalInput")
    w1T = nc.dram_tensor("w1T", [128, 8 * 1024], f16, kind="ExternalInput")
    w2T = nc.dram_tensor("w2T", [128, 8 * 512], f16, kind="ExternalInput")
    w3T = nc.dram_tensor("w3T", [128, 4 * 256], f16, kind="ExternalInput")
    w4T = nc.dram_tensor("w4T", [128, 2], f16, kind="ExternalInput")
    b0 = nc.dram_tensor("b0", [128, 8], f32, kind="ExternalInput")
    b1 = nc.dram_tensor("b1", [128, 8], f32, kind="ExternalInput")
    b2 = nc.dram_tensor("b2", [128, 4], f32, kind="ExternalInput")
    b3 = nc.dram_tensor("b3", [128, 2], f32, kind="ExternalInput")
    b4 = nc.dram_tensor("b4", [1, 1], f32, kind="ExternalInput")

    with tile.TileContext(nc) as tc:
        import contextlib

        with contextlib.ExitStack() as ctx:
            singles = ctx.enter_context(tc.tile_pool(name="singles", bufs=1))
            stage_p = ctx.enter_context(tc.tile_pool(name="stage", bufs=2))
            xt_p = ctx.enter_context(tc.tile_pool(name="xt", bufs=2))
            spart_p = ctx.enter_context(tc.tile_pool(name="spart", bufs=2))
            fi_p = ctx.enter_context(tc.tile_pool(name="fi", bufs=14))
            act_p = ctx.enter_context(tc.tile_pool(name="act", bufs=2))
            out_p = ctx.enter_context(tc.tile_pool(name="out", bufs=2))
            ps_gram = ctx.enter_context(
                tc.tile_pool(name="psgram", bufs=3, space="PSUM"))
            ps_mlp = ctx.enter_context(
                tc.tile_pool(name="psmlp", bufs=4, space="PSUM"))
            ps_l4 = ctx.enter_context(
                tc.tile_pool(name="psl4", bufs=1, space="PSUM"))

            # --- weights to SBUF (once) ---
            w0bT_s = singles.tile([128, 1024], f16)
            nc.sync.dma_start(out=w0bT_s[:], in_=w0bT[:])
            w0p_s = singles.tile([128, _NKT * 1024], f16)
            nc.sync.dma_start(out=w0p_s[:], in_=w0p[:])
            w1T_s = singles.tile([128, 8 * 1024], f16)
            nc.sync.dma_start(out=w1T_s[:], in_=w1T[:])
            w2T_s = singles.tile([128, 8 * 512], f16)
            nc.sync.dma_start(out=w2T_s[:], in_=w2T[:])
            w3T_s = singles.tile([128, 4 * 256], f16)
            nc.sync.dma_start(out=w3T_s[:], in_=w3T[:])
            w4T_s = singles.tile([128, 2], f16)
            nc.sync.dma_start(out=w4T_s[:], in_=w4T[:])
            b0_s = singles.tile([128, 8], f32)
            nc.sync.dma_start(out=b0_s[:], in_=b0[:])
            b1_s = singles.tile([128, 8], f32)
            nc.sync.dma_start(out=b1_s[:], in_=b1[:])
            b2_s = singles.tile([128, 4], f32)
            nc.sync.dma_start(out=b2_s[:], in_=b2[:])
            b3_s = singles.tile([128, 2], f32)
            nc.sync.dma_start(out=b3_s[:], in_=b3[:])
            b4_s = singles.tile([1, 1], f32)
            nc.sync.dma_start(out=b4_s[:], in_=b4[:])

            xap = x[:]

            def front_phase(t):
                # 1. cast-load: [128, g*128] fp16, partition = 32j + n
                stage = stage_p.tile([128, g * 128], f16)
                for j in range(4):
                    src = bass.AP(
                        tensor=xap.tensor,
                        offset=t * nb * _N * _D + j * _N * _D,
                        ap=[[_D, 32], [4 * _N * _D, g], [1, _D]],
                    )
                    nc.gpsimd.dma_start(out=stage[32 * j:32 * j + 32, :],
                                        in_=src)

                # 2. batched xbar transpose -> XT[d, gi*128 + 32j + n]
                xt3 = xt_p.tile([128, g, 128], f16)
                nc.sync.dma_start(out=xt3[:], in_=stage[:], transpose=True)
                xt = xt3[:].rearrange("p g c -> p (g c)")

                # 3+4. gram + extract to S_part[n, s*27+m]
                spart = spart_p.tile([32, nb * _N], f16)
                for b in range(nbanks):
                    ns = min(_SPB, nb - b * _SPB)
                    ps = ps_gram.tile([27, _SPB * _N], f32)
                    for sl in range(ns):
                        s = b * _SPB + sl
                        c0 = (s // 4) * 128 + 32 * (s % 4)
                        nc.tensor.matmul(
                            ps[0:27, sl * _N:(sl + 1) * _N],
                            xt[:, c0:c0 + _N],
                            xt[:, c0:c0 + _N],
                            start=True, stop=True,
                        )
                    nc.scalar.copy(
                        spart[0:27, b * _SPB * _N:(b * _SPB + ns) * _N],
                        ps[0:27, 0:ns * _N],
                    )

                # 5. FI k-tiles
                sp3 = spart.rearrange("p (s m) -> p s m", m=_N)
                fis = []
                for kt in range(_NKT):
                    fi = fi_p.tile([128, nb], f16)
                    nc.vector.memset(fi[:], 0.0)
                    for slot in range(4):
                        m = kt * 4 + slot
                        if m >= _N - 1:
                            continue
                        nc.vector.tensor_copy(
                            fi[slot * 32:slot * 32 + _N, :],
                            sp3[0:27, :, m],
                        )
                    fis.append(fi)

                # F0: bottom row (n=0) of each sample, d-major strided view
                xtap = xt
                f0 = bass.AP(
                    tensor=xtap.tensor,
                    offset=xtap.offset,
                    ap=[xtap.ap[0], [128, g], [32, 4]],
                )
                return f0, fis

            def relu_bias(out, ps, bias):
                nc.vector.tensor_scalar(
                    out, ps, bias, 0.0,
                    mybir.AluOpType.add, mybir.AluOpType.max)

            def mlp_phase(t, f0, fis):
                # 6. MLP
                a1 = act_p.tile([128, 8 * nb], f16, tag="a1")
                for m8 in range(8):
                    ps = ps_mlp.tile([128, nb], f32)
                    nc.tensor.matmul(ps[:], w0bT_s[:, m8 * 128:(m8 + 1) * 128],
                                     f0, start=True, stop=False)
                    for kt in range(_NKT):
                        nc.tensor.matmul(
                            ps[:],
                            w0p_s[:, kt * 1024 + m8 * 128:kt * 1024 + (m8 + 1) * 128],
                            fis[kt][:],
                            start=False, stop=(kt == _NKT - 1),
                        )
                    relu_bias(a1[:, m8 * nb:(m8 + 1) * nb], ps[:], b0_s[:, m8:m8 + 1])

                a2 = act_p.tile([128, 8 * nb], f16, tag="a2")
                for m8 in range(8):
                    ps = ps_mlp.tile([128, nb], f32)
                    for ko in range(8):
                        nc.tensor.matmul(
                            ps[:],
                            w1T_s[:, ko * 1024 + m8 * 128:ko * 1024 + (m8 + 1) * 128],
                            a1[:, ko * nb:(ko + 1) * nb],
                            start=(ko == 0), stop=(ko == 7),
                        )
                    relu_bias(a2[:, m8 * nb:(m8 + 1) * nb], ps[:], b1_s[:, m8:m8 + 1])

                a3 = act_p.tile([128, 4 * nb], f16, tag="a3")
                for m4 in range(4):
                    ps = ps_mlp.tile([128, nb], f32)
                    for ko in range(8):
                        nc.tensor.matmul(
                            ps[:],
                            w2T_s[:, ko * 512 + m4 * 128:ko * 512 + (m4 + 1) * 128],
                            a2[:, ko * nb:(ko + 1) * nb],
                            start=(ko == 0), stop=(ko == 7),
                        )
                    relu_bias(a3[:, m4 * nb:(m4 + 1) * nb], ps[:], b2_s[:, m4:m4 + 1])

                a4 = act_p.tile([128, 2 * nb], f16, tag="a4")
                for m2 in range(2):
                    ps = ps_mlp.tile([128, nb], f32)
                    for ko in range(4):
                        nc.tensor.matmul(
                            ps[:],
                            w3T_s[:, ko * 256 + m2 * 128:ko * 256 + (m2 + 1) * 128],
                            a3[:, ko * nb:(ko + 1) * nb],
                            start=(ko == 0), stop=(ko == 3),
                        )
                    relu_bias(a4[:, m2 * nb:(m2 + 1) * nb], ps[:], b3_s[:, m2:m2 + 1])

                ps4 = ps_l4.tile([1, nb], f32)
                nc.tensor.matmul(ps4[:], w4T_s[:, 0:1], a4[:, 0:nb],
                                 start=True, stop=False)
                nc.tensor.matmul(ps4[:], w4T_s[:, 1:2], a4[:, nb:2 * nb],
                                 start=False, stop=True)
                ov = out_p.tile([1, nb], f32)
                nc.vector.tensor_scalar_add(ov[:], ps4[:], b4_s[0:1, 0:1])
                nc.sync.dma_start(out=y[t * nb:(t + 1) * nb, :], in_=ov[:])

            prev = None
            for t in range(nt):
                ctx_t = front_phase(t)
                if prev is not None:
                    mlp_phase(t - 1, *prev)
                prev = ctx_t
            mlp_phase(nt - 1, *prev)

    nc.compile()
    return nc


def _prep_weights(W0, b0, W1, b1, W2, b2, W3, b3, W4, b4):
    f16 = np.float16
    tr, tc_ = np.tril_indices(_N, k=-1)
    w0p = np.zeros((128, _NKT * 1024), dtype=f16)
    for p, (n, m) in enumerate(zip(tr, tc_)):
        kt, slot = m // 4, m % 4
        w0p[slot * 32 + n, kt * 1024:(kt + 1) * 1024] = W0[:, 128 + p].astype(f16)

    def pack(WT, mdim, ktiles):
        # WT [K, M] -> [128, ktiles*M]
        K, M = WT.shape
        return (WT.reshape(ktiles, 128, M).transpose(1, 0, 2)
                .reshape(128, ktiles * M).astype(f16))

    return {
        "w0bT": np.ascontiguousarray(W0[:, :128].T).astype(f16),
        "w0p": w0p,
        "w1T": pack(W1.T, 1024, 8),
        "w2T": pack(W2.T, 512, 8),
        "w3T": pack(W3.T, 256, 4),
        "w4T": pack(W4.T, 1, 2),
        "b0": np.ascontiguousarray(b0.reshape(8, 128).T).astype(np.float32),
        "b1": np.ascontiguousarray(b1.reshape(8, 128).T).astype(np.float32),
        "b2": np.ascontiguousarray(b2.reshape(4, 128).T).astype(np.float32),
        "b3": np.ascontiguousarray(b3.reshape(2, 128).T).astype(np.float32),
        "b4": np.array([[b4[0]]], dtype=np.float32),
    }


def kernel(**inputs):
    from concourse.bass_utils import run_bass_kernel_spmd

    x = np.asarray(inputs["bottom_output"], dtype=np.float32)
    B = x.shape[0]
    bc = B // _NCORES
    nt = bc // _NB
    key = (_NB, nt)
    if key not in _compiled:
        _compiled[key] = _build(_NB, nt)
    nc = _compiled[key]

    wmap = _prep_weights(
        np.asarray(inputs["W0"]), np.asarray(inputs["b0"]),
        np.asarray(inputs["W1"]), np.asarray(inputs["b1"]),
        np.asarray(inputs["W2"]), np.asarray(inputs["b2"]),
        np.asarray(inputs["W3"]), np.asarray(inputs["b3"]),
        np.asarray(inputs["W4"]), np.asarray(inputs["b4"]),
    )

    in_maps = []
    for i in range(_NCORES):
        shard = x[i * bc:(i + 1) * bc]
        xflat = np.concatenate(
            [shard.reshape(-1), np.zeros(8 * _D, dtype=np.float32)])
        m = {"x": xflat}
        m.update(wmap)
        in_maps.append(m)

    res = run_bass_kernel_spmd(nc, in_maps, list(range(_NCORES)))
    out = np.concatenate([res.results[i]["y"] for i in range(_NCORES)], axis=0)
    return out.astype(np.float32)


# revision 7
# speedup vs baseline: 1.0708x; 1.0708x over previous
"""DLRM-top kernel for 8 TRN2 NeuronCores (data-parallel over batch).

Pipeline per core (4096 samples, tiles of NB):
  1. gpsimd cast-DMA loads x tile f32->fp16 into stage [112, G*128]
     (partition = 32*j + n for 4 samples j per group, pitch-32 junk rows).
  2. xbar DMA-transpose per group: stage[:, g*128:+128] -> XT[:, g*128:+128]
     giving d-major layout XT[d, g*128 + 32j + n] = x[4g+j, n, d].
  3. Per-sample gram matmuls (fp16): psum[0:27, sl*27:+27] = Xs @ Xs.T.
  4. ACT copies psum -> S_part[n, s*27 + m] (fp16).
  5. DVE copies build FI k-tiles [128, NB]: pair (m, n>m) at partition
     (m%4)*32 + n of k-tile m//4 (garbage rows masked by zero weights).
  6. Feature-major MLP, batch on free dim (N=NB), fp16 weights with FWL.
"""

import sys

sys.path.insert(0, "/opt/trn_rl_repo")

import numpy as np

_B = 32768
_NCORES = 8
_BC = _B // _NCORES  # 4096
_NB = 256  # samples per tile
_G = _NB // 4  # transpose groups per tile
_N = 27
_D = 128
_NPAIR = _N * (_N - 1) // 2  # 351
_NKT = 7  # FI k-tiles (27 m-blocks in 32-row slots, 4 per tile)
_SPB = 18  # gram samples per psum bank (18*27=486 <= 512)

_compiled = {}


def _build(nb, nt):
    import concourse.bass as bass
    import concourse.mybir as mybir
    import concourse.tile as tile
    from concourse import bacc

    f16 = mybir.dt.float16
    f32 = mybir.dt.float32
    Relu = mybir.ActivationFunctionType.Relu

    g = nb // 4
    nbanks = (nb + _SPB - 1) // _SPB  # gram psum banks per tile

    nc = bacc.Bacc("TRN2", target_bir_lowering=False, debug=False,
                   num_devices=_NCORES)

    x = nc.dram_tensor("x", [nb * nt * _N * _D + 8 * _D], f32, kind="ExternalInput")
    y = nc.dram_tensor("y", [nb * nt, 1], f32, kind="ExternalOutput")
    w0bT = nc.dram_tensor("w0bT", [128, 1024], f16, kind="ExternalInput")
    w0p = nc.dram_tensor("w0p", [128, _NKT * 1024], f16, kind="ExternalInput")
    w1T = nc.dram_tensor("w1T", [128, 8 * 1024], f16, kind="ExternalInput")
    w2T = nc.dram_tensor("w2T", [128, 8 * 512], f16, kind="ExternalInput")
    w3T = nc.dram_tensor("w3T", [128, 4 * 256], f16, kind="ExternalInput")
    w4T = nc.dram_tensor("w4T", [128, 2], f16, kind="ExternalInput")
    b0 = nc.dram_tensor("b0", [128, 8], f32, kind="ExternalInput")
    b1 = nc.dram_tensor("b1", [128, 8], f32, kind="ExternalInput")
    b2 = nc.dram_tensor("b2", [128, 4], f32, kind="ExternalInput")
    b3 = nc.dram_tensor("b3", [128, 2], f32, kind="ExternalInput")
    b4 = nc.dram_tensor("b4", [1, 1], f32, kind="ExternalInput")

    with tile.TileContext(nc) as tc:
        import contextlib

        with contextlib.ExitStack() as ctx:
            singles = ctx.enter_context(tc.tile_pool(name="singles", bufs=1))
            stage_p = ctx.enter_context(tc.tile_pool(name="stage", bufs=2))
            xt_p = ctx.enter_context(tc.tile_pool(name="xt", bufs=2))
            spart_p = ctx.enter_context(tc.tile_pool(name="spart", bufs=2))
            fi_p = ctx.enter_context(tc.tile_pool(name="fi", bufs=14))
            act_p = ctx.enter_context(tc.tile_pool(name="act", bufs=2))
            out_p = ctx.enter_context(tc.tile_pool(name="out", bufs=2))
            ps_gram = ctx.enter_context(
                tc.tile_pool(name="psgram", bufs=3, space="PSUM"))
            ps_mlp = ctx.enter_context(
                tc.tile_pool(name="psmlp", bufs=4, space="PSUM"))
            ps_l4 = ctx.enter_context(
                tc.tile_pool(name="psl4", bufs=1, space="PSUM"))

            # --- weights to SBUF (once) ---
            w0bT_s = singles.tile([128, 1024], f16)
            nc.scalar.dma_start(out=w0bT_s[:], in_=w0bT[:])
            w0p_s = singles.tile([128, _NKT * 1024], f16)
            nc.scalar.dma_start(out=w0p_s[:], in_=w0p[:])
            w1T_s = singles.tile([128, 8 * 1024], f16)
            nc.scalar.dma_start(out=w1T_s[:], in_=w1T[:])
            w2T_s = singles.tile([128, 8 * 512], f16)
            nc.scalar.dma_start(out=w2T_s[:], in_=w2T[:])
            w3T_s = singles.tile([128, 4 * 256], f16)
            nc.scalar.dma_start(out=w3T_s[:], in_=w3T[:])
            w4T_s = singles.tile([128, 2], f16)
            nc.scalar.dma_start(out=w4T_s[:], in_=w4T[:])
            b0_s = singles.tile([128, 8], f32)
            nc.scalar.dma_start(out=b0_s[:], in_=b0[:])
            b1_s = singles.tile([128, 8], f32)
            nc.scalar.dma_start(out=b1_s[:], in_=b1[:])
            b2_s = singles.tile([128, 4], f32)
            nc.scalar.dma_start(out=b2_s[:], in_=b2[:])
            b3_s = singles.tile([128, 2], f32)
            nc.scalar.dma_start(out=b3_s[:], in_=b3[:])
            b4_s = singles.tile([1, 1], f32)
            nc.scalar.dma_start(out=b4_s[:], in_=b4[:])

            xap = x[:]

            def front_phase(t):
                # 1. cast-load: [128, g*128] fp16, partition = 32j + n
                stage = stage_p.tile([128, g * 128], f16)
                for j in range(4):
                    src = bass.AP(
                        tensor=xap.tensor,
                        offset=t * nb * _N * _D + j * _N * _D,
                        ap=[[_D, 32], [4 * _N * _D, g], [1, _D]],
                    )
                    nc.gpsimd.dma_start(out=stage[32 * j:32 * j + 32, :],
                                        in_=src)

                # 2. batched xbar transpose -> XT[d, gi*128 + 32j + n]
                xt3 = xt_p.tile([128, g, 128], f16)
                nc.sync.dma_start(out=xt3[:], in_=stage[:], transpose=True)
                xt = xt3[:].rearrange("p g c -> p (g c)")

                # 3+4. gram + extract to S_part[n, s*27+m]
                spart = spart_p.tile([32, nb * _N], f16)
                for b in range(nbanks):
                    ns = min(_SPB, nb - b * _SPB)
                    ps = ps_gram.tile([27, _SPB * _N], f32)
                    for sl in range(ns):
                        s = b * _SPB + sl
                        c0 = (s // 4) * 128 + 32 * (s % 4)
                        nc.tensor.matmul(
                            ps[0:27, sl * _N:(sl + 1) * _N],
                            xt[:, c0:c0 + _N],
                            xt[:, c0:c0 + _N],
                            start=True, stop=True,
                        )
                    nc.scalar.copy(
                        spart[0:27, b * _SPB * _N:(b * _SPB + ns) * _N],
                        ps[0:27, 0:ns * _N],
                    )

                # 5. FI k-tiles
                sp3 = spart.rearrange("p (s m) -> p s m", m=_N)
                fis = []
                for kt in range(_NKT):
                    fi = fi_p.tile([128, nb], f16)
                    nc.vector.memset(fi[:], 0.0)
                    for slot in range(4):
                        m = kt * 4 + slot
                        if m >= _N - 1:
                            continue
                        nc.vector.tensor_copy(
                            fi[slot * 32:slot * 32 + _N, :],
                            sp3[0:27, :, m],
                        )
                    fis.append(fi)

                # F0: bottom row (n=0) of each sample, d-major strided view
                xtap = xt
                f0 = bass.AP(
                    tensor=xtap.tensor,
                    offset=xtap.offset,
                    ap=[xtap.ap[0], [128, g], [32, 4]],
                )
                return f0, fis

            def relu_bias(out, ps, bias):
                nc.scalar.activation(out, ps, Relu, bias=bias)

            def mlp_phase(t, f0, fis):
                # 6. MLP
                a1 = act_p.tile([128, 8 * nb], f16, tag="a1")
                for m8 in range(8):
                    ps = ps_mlp.tile([128, nb], f32)
                    nc.tensor.matmul(ps[:], w0bT_s[:, m8 * 128:(m8 + 1) * 128],
                                     f0, start=True, stop=False)
                    for kt in range(_NKT):
                        nc.tensor.matmul(
                            ps[:],
                            w0p_s[:, kt * 1024 + m8 * 128:kt * 1024 + (m8 + 1) * 128],
                            fis[kt][:],
                            start=False, stop=(kt == _NKT - 1),
                        )
                    relu_bias(a1[:, m8 * nb:(m8 + 1) * nb], ps[:], b0_s[:, m8:m8 + 1])

                a2 = act_p.tile([128, 8 * nb], f16, tag="a2")
                for m8 in range(8):
                    ps = ps_mlp.tile([128, nb], f32)
                    for ko in range(8):
                        nc.tensor.matmul(
                            ps[:],
                            w1T_s[:, ko * 1024 + m8 * 128:ko * 1024 + (m8 + 1) * 128],
                            a1[:, ko * nb:(ko + 1) * nb],
                            start=(ko == 0), stop=(ko == 7),
                        )
                    relu_bias(a2[:, m8 * nb:(m8 + 1) * nb], ps[:], b1_s[:, m8:m8 + 1])

                a3 = act_p.tile([128, 4 * nb], f16, tag="a3")
                for m4 in range(4):
                    ps = ps_mlp.tile([128, nb], f32)
                    for ko in range(8):
                        nc.tensor.matmul(
                            ps[:],
                            w2T_s[:, ko * 512 + m4 * 128:ko * 512 + (m4 + 1) * 128],
                            a2[:, ko * nb:(ko + 1) * nb],
                            start=(ko == 0), stop=(ko == 7),
                        )
                    relu_bias(a3[:, m4 * nb:(m4 + 1) * nb], ps[:], b2_s[:, m4:m4 + 1])

                a4 = act_p.tile([128, 2 * nb], f16, tag="a4")
                for m2 in range(2):
                    ps = ps_mlp.tile([128, nb], f32)
                    for ko in range(4):
                        nc.tensor.matmul(
                            ps[:],
                            w3T_s[:, ko * 256 + m2 * 128:ko * 256 + (m2 + 1) * 128],
                            a3[:, ko * nb:(ko + 1) * nb],
                            start=(ko == 0), stop=(ko == 3),
                        )
                    relu_bias(a4[:, m2 * nb:(m2 + 1) * nb], ps[:], b3_s[:, m2:m2 + 1])

                ps4 = ps_l4.tile([1, nb], f32)
                nc.tensor.matmul(ps4[:], w4T_s[:, 0:1], a4[:, 0:nb],
                                 start=True, stop=False)
                nc.tensor.matmul(ps4[:], w4T_s[:, 1:2], a4[:, nb:2 * nb],
                                 start=False, stop=True)
                ov = out_p.tile([1, nb], f32)
                nc.vector.tensor_scalar_add(ov[:], ps4[:], b4_s[0:1, 0:1])
                nc.sync.dma_start(out=y[t * nb:(t + 1) * nb, :], in_=ov[:])

            prev = None
            for t in range(nt):
                ctx_t = front_phase(t)
                if prev is not None:
                    mlp_phase(t - 1, *prev)
                prev = ctx_t
            mlp_phase(nt - 1, *prev)

    nc.compile()
    return nc


def _prep_weights(W0, b0, W1, b1, W2, b2, W3, b3, W4, b4):
    f16 = np.float16
    tr, tc_ = np.tril_indices(_N, k=-1)
    w0p = np.zeros((128, _NKT * 1024), dtype=f16)
    for p, (n, m) in enumerate(zip(tr, tc_)):
        kt, slot = m // 4, m % 4
        w0p[slot * 32 + n, kt * 1024:(kt + 1) * 1024] = W0[:, 128 + p].astype(f16)

    def pack(WT, mdim, ktiles):
        # WT [K, M] -> [128, ktiles*M]
        K, M = WT.shape
        return (WT.reshape(ktiles, 128, M).transpose(1, 0, 2)
                .reshape(128, ktiles * M).astype(f16))

    return {
        "w0bT": np.ascontiguousarray(W0[:, :128].T).astype(f16),
        "w0p": w0p,
        "w1T": pack(W1.T, 1024, 8),
        "w2T": pack(W2.T, 512, 8),
        "w3T": pack(W3.T, 256, 4),
        "w4T": pack(W4.T, 1, 2),
        "b0": np.ascontiguousarray(b0.reshape(8, 128).T).astype(np.float32),
        "b1": np.ascontiguousarray(b1.reshape(8, 128).T).astype(np.float32),
        "b2": np.ascontiguousarray(b2.reshape(4, 128).T).astype(np.float32),
        "b3": np.ascontiguousarray(b3.reshape(2, 128).T).astype(np.float32),
        "b4": np.array([[b4[0]]], dtype=np.float32),
    }


def kernel(**inputs):
    from concourse.bass_utils import run_bass_kernel_spmd

    x = np.asarray(inputs["bottom_output"], dtype=np.float32)
    B = x.shape[0]
    bc = B // _NCORES
    nt = bc // _NB
    key = (_NB, nt)
    if key not in _compiled:
        _compiled[key] = _build(_NB, nt)
    nc = _compiled[key]

    wmap = _prep_weights(
        np.asarray(inputs["W0"]), np.asarray(inputs["b0"]),
        np.asarray(inputs["W1"]), np.asarray(inputs["b1"]),
        np.asarray(inputs["W2"]), np.asarray(inputs["b2"]),
        np.asarray(inputs["W3"]), np.asarray(inputs["b3"]),
        np.asarray(inputs["W4"]), np.asarray(inputs["b4"]),
    )

    in_maps = []
    for i in range(_NCORES):
        shard = x[i * bc:(i + 1) * bc]
        xflat = np.concatenate(
            [shard.reshape(-1), np.zeros(8 * _D, dtype=np.float32)])
        m = {"x": xflat}
        m.update(wmap)
        in_maps.append(m)

    res = run_bass_kernel_spmd(nc, in_maps, list(range(_NCORES)))
    out = np.concatenate([res.results[i]["y"] for i in range(_NCORES)], axis=0)
    return out.astype(np.float32)
